# revision 11
# baseline (speedup 1.0000x reference)
"""Trainium2 Bass kernel for nn_MinBlcokScan: 4 grouped 1-D cross-correlations.

Math (reference): x = batch_x.reshape(B, 32, L). For each group g of 4,
channels rel_g = [8g..8g+7] are convolved ('same', zero pad 2/2) with
kernels_g [4, 8, 5], producing out[:, 4g+o, :]; the 16 output channels are
concatenated and flattened to [B, 16*L].

Strategy: pure data parallel over batch (4 samples per core). The conv is
memory-bound, so x and y stream as fp16 (host-side cast is free; rel-err
budget 2e-2 vs fp16's ~3e-4) with a split-parity polyphase-8 layout that
fills the full 128x128 PE array:

Host-side marshalling (free for the device):
  x_pad[c, i] = x[c, i-2] (zero pad, length L+8), split into even/odd
  half-columns of 4 consecutive positions:
    x_e[(c,p), m] = x_pad[c, 8m + p]       p in [0,4), m in [0, L/8]
    x_o[(c,p), m] = x_pad[c, 8m + 4 + p]   p in [0,4), m in [0, L/8)
  Output column m carries 8 positions (r' in [0,8)); its receptive field
  [8m-2, 8m+9] is exactly x_e[:, m] U x_o[:, m] U x_e[:, m+1], so each
  PSUM tile accumulates 3 matmuls with 128-partition outputs:
    y[o, 8m+r'] = (We0.T x_e[:,m] + Wo.T x_o[:,m] + We1.T x_e[:,m+1])[(o,r')]
    We0[(c,p),(o,r')] = ker[o,c, p-r']      (offset v = p-2  in [-2,2))
    Wo [(c,p),(o,r')] = ker[o,c, p+4-r']    (offset v = p+2  in [2,6))
    We1[(c,p),(o,r')] = ker[o,c, p+8-r']    (offset v = p+6  in [6,10))
  (entries with tap index outside [0,5) are zero). Output is produced
  parity-interleaved y_i[(o,r'), m] = y[o, 8m+r'] in fp16 and
  de-interleaved/upcast on the host.

Per core: PE streams 3 * L/8 * 4 = 1.5L columns (~41 us at 2.4 GHz), DMA
moves 16.8 MB in + 8.4 MB out = 25.2 MB (~70 us at ~355 GB/s) -> DMA-bound.
PSUM->SBUF drains are full-width [128, 2048] copies alternating between
the DVE and Activation engines.
"""

import numpy as np
from contextlib import ExitStack

import concourse.bass as bass
import concourse.bacc as bacc
import concourse.mybir as mybir
import concourse.tile as tile
from concourse.bass_utils import run_bass_kernel_spmd

D = 32          # input channels
L = 65536       # sequence length
W = 5           # conv window
B = 32          # batch
N_CORES = 8
S = 4           # samples per core
MO = L // 8     # 8192 output columns per sample
ME = MO + 1     # stored even x columns per sample (one halo column)
MC = ME + MO    # per-sample x segment: [x_e | x_o] = 16385 columns
NSUB = 512      # one fp32 PSUM bank = 512 columns at 128 partitions
F16 = mybir.dt.float16
F32 = mybir.dt.float32


def build_program(variant="full"):
    """Build the single-core SPMD Bass program (same program on all cores).

    Work unit: "super-block" of SB=4096 output columns (one xe/xo DMA pair
    with 8 KB/partition descriptors), computed as sub-blocks of 2048 cols
    (one [128, 2048] PSUM tile = 4 banks, weight-outer: 3 stationary loads
    x 4 matmuls). The final super-block tapers into 2048/1024/512/256/256
    sub-blocks with per-sub loads and stores (from dedicated tile pools so
    nothing is pool-gated) to shorten the pipeline drain.
    variant: "full" | "dma" (loads+stores only) | "pe" (loads+matmuls only)
    """
    SB = 4096
    nsb = MO // SB  # super-blocks per sample

    nc = bacc.Bacc(trn_type="TRN2", target_bir_lowering=False, debug=False)
    x = nc.dram_tensor("x", [128, S * MC], F16, kind="ExternalInput").ap()
    w = nc.dram_tensor("w", [128, 3 * 128], F16, kind="ExternalInput").ap()
    y = nc.dram_tensor("y", [128, S * MO], F16, kind="ExternalOutput").ap()

    with tile.TileContext(nc) as tc, ExitStack() as ctx:
        xep = ctx.enter_context(tc.tile_pool(name="xep", bufs=3))
        xop = ctx.enter_context(tc.tile_pool(name="xop", bufs=3))
        # dedicated pools for the final tapered super-block: all 5 sub
        # tiles coexist, so their DMAs issue back-to-back (no pool gating)
        xet = ctx.enter_context(tc.tile_pool(name="xet", bufs=5))
        xot = ctx.enter_context(tc.tile_pool(name="xot", bufs=5))
        opt = ctx.enter_context(tc.tile_pool(name="opt", bufs=5))
        wp = ctx.enter_context(tc.tile_pool(name="wp", bufs=1))
        op = ctx.enter_context(tc.tile_pool(name="op", bufs=3))
        # [128, 2048] fp32 = 4 banks per tile; 2 tiles fill all 8 banks
        pp = ctx.enter_context(tc.tile_pool(name="pp", bufs=2, space="PSUM"))

        wt = wp.tile([128, 3 * 128], F16)
        nc.sync.dma_start(wt[:], w)

        copy_engines = [nc.vector.tensor_copy, nc.scalar.copy]
        sub_idx = 0
        for s in range(S):
            e0 = s * MC            # even columns base
            o0 = s * MC + ME       # odd columns base
            for b in range(nsb):
                m0 = b * SB
                last = (s == S - 1) and (b == nsb - 1)

                if not last:
                    xe = xep.tile([128, SB + 1], F16)
                    nc.sync.dma_start(xe[:], x[:, e0 + m0 : e0 + m0 + SB + 1])
                    xo = xop.tile([128, SB], F16)
                    nc.sync.dma_start(xo[:], x[:, o0 + m0 : o0 + m0 + SB])

                    ot = None
                    if variant != "pe":
                        ot = op.tile([128, SB], F16)
                    if variant == "dma":
                        nc.vector.memset(ot[:], 0.0)
                        nc.scalar.dma_start(
                            y[:, s * MO + m0 : s * MO + m0 + SB], ot[:]
                        )
                        continue

                    c0 = 0  # sub-block offset within the super-block
                    for sub in (2048, 2048):
                        pt = pp.tile([128, sub], F32)
                        for j in range(3):  # weight-outer stationary reuse
                            xsrc = xo if j == 1 else xe
                            off = c0 + (1 if j == 2 else 0)
                            for q in range(sub // NSUB):
                                nc.tensor.matmul(
                                    pt[:, q * NSUB : (q + 1) * NSUB],
                                    wt[:, j * 128 : (j + 1) * 128],
                                    xsrc[:, off + q * NSUB : off + q * NSUB + NSUB],
                                    start=(j == 0),
                                    stop=(j == 2),
                                )
                        if variant == "full":
                            copy_engines[sub_idx % 2](ot[:, c0 : c0 + sub], pt[:])
                        sub_idx += 1
                        c0 += sub

                    if variant != "pe":
                        y0 = s * MO + m0
                        nc.scalar.dma_start(y[:, y0 : y0 + SB], ot[:])
                else:
                    # Final super-block: tapered sub-blocks with per-sub
                    # input DMAs and stores, so the drain after the last
                    # input byte is only one short sub-block's chain.
                    c0 = 0
                    for sub in (2048, 1024, 512, 256, 256):
                        xe = xet.tile([128, sub + 1], F16)
                        nc.sync.dma_start(
                            xe[:], x[:, e0 + m0 + c0 : e0 + m0 + c0 + sub + 1]
                        )
                        xo = xot.tile([128, sub], F16)
                        nc.sync.dma_start(
                            xo[:], x[:, o0 + m0 + c0 : o0 + m0 + c0 + sub]
                        )
                        ot = None
                        if variant != "pe":
                            ot = opt.tile([128, sub], F16)
                        if variant == "dma":
                            nc.vector.memset(ot[:], 0.0)
                            y0 = s * MO + m0 + c0
                            nc.scalar.dma_start(y[:, y0 : y0 + sub], ot[:])
                            c0 += sub
                            continue
                        pt = pp.tile([128, sub], F32)
                        for j in range(3):
                            xsrc = xo if j == 1 else xe
                            off = 1 if j == 2 else 0
                            for q0 in range(0, sub, NSUB):
                                qw = min(NSUB, sub - q0)
                                nc.tensor.matmul(
                                    pt[:, q0 : q0 + qw],
                                    wt[:, j * 128 : (j + 1) * 128],
                                    xsrc[:, off + q0 : off + q0 + qw],
                                    start=(j == 0),
                                    stop=(j == 2),
                                )
                        if variant == "full":
                            copy_engines[sub_idx % 2](ot[:], pt[:])
                            y0 = s * MO + m0 + c0
                            nc.scalar.dma_start(y[:, y0 : y0 + sub], ot[:])
                        sub_idx += 1
                        c0 += sub
    nc.compile()
    return nc


def build_weights(kernels):
    """W [128, 3*128]: W[(c,p), j*128 + (o,r')] = ker_g[o, c, t] where
    t = p - r' (j=0, x_e), p + 4 - r' (j=1, x_o), p + 8 - r' (j=2, x_e+1)."""
    Wd = np.zeros((3, 128, 128), np.float32)
    for g, ker in enumerate(kernels):  # ker [4, 8, 5]
        for oo in range(4):
            o = 4 * g + oo
            for cc in range(8):
                c = 8 * g + cc
                for rp in range(8):
                    for p in range(4):
                        for j in range(3):
                            t = 4 * j + p - rp
                            if 0 <= t < W:
                                Wd[j, c * 4 + p, o * 8 + rp] = ker[oo, cc, t]
    return np.ascontiguousarray(
        np.concatenate([Wd[0], Wd[1], Wd[2]], axis=1)
    ).astype(np.float16)


def interleave_x(x1, dtype=np.float16):
    """[32, L] -> [128, MC]: per-sample [x_e | x_o] split-parity layout."""
    xp = np.zeros((D, L + 8), np.float32)
    xp[:, 2 : L + 2] = x1
    xr = xp.reshape(D, ME, 8)  # ME * 8 = L + 8
    xe = xr[:, :, 0:4].transpose(0, 2, 1).reshape(D * 4, ME)
    xo = xr[:, :MO, 4:8].transpose(0, 2, 1).reshape(D * 4, MO)
    return np.ascontiguousarray(
        np.concatenate([xe, xo], axis=1).astype(dtype)
    )


def deinterleave_y(yi):
    """[128, S*MO] fp16 -> [S*16, L] fp32: yi[o*8+r', s*MO+m] = y[s,o,8m+r']."""
    t = yi.reshape(16, 8, S, MO).transpose(2, 0, 3, 1)  # s, o, m, r'
    return np.ascontiguousarray(t.astype(np.float32).reshape(S * 16, L))


_program_cache = {}

# Set PROFILE=True (e.g. from a test harness) to capture an NTFF profile;
# the BassKernelResults lands in LAST_RESULT.
PROFILE = False
PROFILE_TMPDIR = None
LAST_RESULT = None


def kernel(batch_x, kernels0, kernels1, kernels2, kernels3):
    global LAST_RESULT
    batch_x = np.asarray(batch_x)
    kernels = [np.asarray(k) for k in (kernels0, kernels1, kernels2, kernels3)]
    Wd = build_weights(kernels)

    if "nc" not in _program_cache:
        _program_cache["nc"] = build_program()
    nc = _program_cache["nc"]

    in_maps = []
    for k in range(N_CORES):
        xs = [
            interleave_x(batch_x[S * k + s].reshape(D, L)) for s in range(S)
        ]
        in_maps.append({"x": np.concatenate(xs, axis=1), "w": Wd})

    res = run_bass_kernel_spmd(
        nc, in_maps, list(range(N_CORES)), trace=PROFILE, tmpdir=PROFILE_TMPDIR
    )
    LAST_RESULT = res
    ys = [deinterleave_y(res.results[k]["y"]) for k in range(N_CORES)]
    return np.concatenate(ys, axis=0).reshape(B, 16 * L)
